# revision 21
# baseline (speedup 1.0000x reference)
"""CrossAttnBlock TRN2 kernel: 8-way (batch x l-half) sharded, collective-free.

Reference math (b=4, c=64, h=64, w=32, dim=256, HEADS=8, l=h*w=2048):
  zf = z.reshape(b, dim, l).T            # [b, l, dim]
  q  = x.reshape(b, c, l).T              # [b, l, c]
  k  = (zf @ Wk + bk) -> [b, H, l, c];  v likewise
  S  = q @ k.T / sqrt(c); A = softmax(S, -1); P = A @ v
  out = (P heads-concat) @ Wo + bo       # [b, l, c]
  return x + out.reshape(b, c, h, w)     # raw-memory reinterpretation

Per-core (core = bi*2 + half): full K/V projection for batch bi, attention +
out-proj for l rows [half*1024, (half+1)*1024).

v2 structure (vs v1): score matmuls write paired 2-bank PSUM tiles so ONE
exp op per iteration covers both heads of the pair; the exp rotates across
Act / Pool / DVE (weighted) instead of saturating Act+DVE only. Phase A/B
drains are paired the same way. Phase D (normalize via recip broadcast +
strided head-reduce + residual) is emitted per-128-row subtile, interleaved
into the second l-half's attention so the tail is only the last 4 subtiles.
Input DMAs are chunked and split across the SP and Act DGE queues so phase A
starts ~1.5us in. Softmax denominators come from a ones-augmented V column
(memset, not DMA'd). bo and bv fold into the host-side residual; bk is added
on the kT drain path.
"""
import ml_dtypes
import numpy as np

import concourse.bass as bass
import concourse.mybir as mybir
import concourse.tile as tile
from concourse import bacc
from concourse.bass_utils import run_bass_kernel_spmd

F32 = mybir.dt.float32
BF16 = mybir.dt.bfloat16
I16 = mybir.dt.int16
I8 = mybir.dt.int8
F8 = mybir.dt.float8e4

B, C, H, W = 4, 64, 64, 32
DIM = 256
HEADS = 8
L = H * W            # 2048
LH = L // 2          # 1024 per core
INNER = HEADS * C    # 512
N_CORES = 8
NMT = L // 128       # 16 m-tiles
NMP = NMT // 2       # 8 m-tile pairs
NLS = LH // 128      # 8 l-subtiles
NP = HEADS // 2      # 4 head pairs

SCALE = float(C) ** -0.5
# Schraudolph exp in fp8e4m3 bits: fp8(exp(s*SCALE)) ~ int8(s*EXP_A8 + EXP_B8)
EXP_A8 = (2.0 ** 3 / float(np.log(2.0))) * SCALE
EXP_B8 = 56.0 - 0.46

# exp engine per iteration (mod len): A=Act table exp, D=DVE schraudolph.
# (GPSIMD/Pool cannot access PSUM, so it only gets SBUF-side phase-D work.)
# Ratio tuned from measured pair-op costs: Act 1058ns vs DVE 1225ns, with
# Act also carrying the pt drains + po copies and DVE the A/B drains.
EXP_ROT = "AD"
# AV matmuls trail the score matmuls by this many m-tile pairs so the PE
# never waits on a just-issued exp
TRAIL = 2

_CACHE = {}


def build_nc():
    nc = bacc.Bacc("TRN2", target_bir_lowering=False, debug=False,
                   num_devices=N_CORES)
    # weights/activations arrive host-pre-interleaved in the DoubleRow
    # [partition, row-pair, col] layout so each DMA is one contiguous run
    # per partition (128 big descriptors instead of 256 small ones)
    zb = nc.dram_tensor("zb", [128, 2, L], F8, kind="ExternalInput")
    Wk = nc.dram_tensor("Wk", [128, 2, INNER], F8, kind="ExternalInput")
    Wv = nc.dram_tensor("Wv", [128, 2, INNER], F8, kind="ExternalInput")
    xq = nc.dram_tensor("xq", [C, LH], BF16, kind="ExternalInput")
    xr = nc.dram_tensor("xr", [128, NLS, C], F32, kind="ExternalInput")
    Wo = nc.dram_tensor("Wo", [C, HEADS, C], BF16, kind="ExternalInput")
    bk = nc.dram_tensor("bk", [128, 4], F32, kind="ExternalInput")
    OUT = nc.dram_tensor("out", [LH, C], F32, kind="ExternalOutput")

    AF = mybir.ActivationFunctionType
    OP = mybir.AluOpType
    DR = mybir.MatmulPerfMode.DoubleRow

    with tile.TileContext(nc) as tc:
        with (
            tc.tile_pool(name="const", bufs=1) as cp,
            tc.tile_pool(name="es", bufs=5) as ep,
            tc.tile_pool(name="sm", bufs=3) as sp,
            tc.tile_pool(name="ps_s", bufs=3, space="PSUM") as ps_s,
            tc.tile_pool(name="ps_pt", bufs=2, space="PSUM") as ps_pt,
        ):
            # ---- inputs to SBUF. sync (SP) queue: wk whole (first matmul
            # needs it), z whole (contiguous beats chunked: one 630ns
            # trigger, full-rate transfer), bk, wv.
            wk_sb = cp.tile([128, 2, INNER], F8, tag="wk")
            nc.sync.dma_start(out=wk_sb, in_=Wk[:, :, :])
            z_sb = cp.tile([128, 2, L], F8, tag="z")
            nc.sync.dma_start(out=z_sb, in_=zb[:, :, :])
            bk_sb = cp.tile([128, 4], F32, tag="bk")
            nc.sync.dma_start(out=bk_sb, in_=bk[:, :])
            wv_sb = cp.tile([128, 2, INNER], F8, tag="wv")
            nc.sync.dma_start(out=wv_sb, in_=Wv[:, :, :])
            # scalar (Act) queue: x (q), residual, Wo
            x_sb = cp.tile([128, LH], BF16, tag="x")
            nc.scalar.dma_start(out=x_sb[0:C, :], in_=xq[:, :])
            nc.scalar.dma_start(out=x_sb[C:2 * C, :], in_=xq[:, :])
            xr_sb = cp.tile([128, NLS, C], F32, tag="xr")
            nc.scalar.dma_start(out=xr_sb, in_=xr[:, :, :])
            wo_sb = cp.tile([C, HEADS, C], BF16, tag="wo")
            nc.scalar.dma_start(out=wo_sb, in_=Wo[:, :, :])

            kT = [cp.tile([128, L], BF16, tag=f"kT{t}", name=f"kT{t}")
                  for t in range(4)]
            v_sb = cp.tile([128, NMT, HEADS, C + 2], F8, tag="v")
            nc.gpsimd.memset(v_sb[:, :, :, C:C + 1], 1.0)
            # P^T numerators for all heads: [c+1, head, l]; row C holds the
            # softmax denominators
            pt_all = cp.tile([C + 1, HEADS, LH], BF16, tag="pt")
            # denominators, rows 0-7 used; padded to 32 partitions for the
            # DVE block transpose
            sums_sb = cp.tile([32, LH], BF16, tag="sums")
            nc.gpsimd.memset(sums_sb, 1.0)

            def drain_engine(idx, out, in_, bias=None):
                # alternate Act/DVE: both are otherwise idle in the prologue
                if bias is not None:
                    if idx % 2 == 0:
                        nc.scalar.activation(out=out, in_=in_,
                                             func=AF.Identity, bias=bias,
                                             scale=1.0)
                    else:
                        nc.vector.tensor_scalar(out=out, in0=in_,
                                                scalar1=bias, scalar2=None,
                                                op0=OP.add)
                else:
                    if idx % 2 == 0:
                        nc.scalar.activation(out=out, in_=in_, func=AF.Copy)
                    else:
                        nc.vector.tensor_copy(out=out, in_=in_)

            # ---- Phase A: kT[t][ci, m] = (Wk^T @ zf^T) + bk, s-paired ----
            for t in range(4):
                for spr in range(2):
                    pk = ps_s.tile([128, 2, 512], F32, tag="s", name="pk")
                    for k in range(2):
                        s = 2 * spr + k
                        nc.tensor.matmul(
                            pk[:, k, :],
                            wk_sb[:, :, t * 128:(t + 1) * 128],
                            z_sb[:, :, s * 512:(s + 1) * 512],
                            start=True, stop=True, perf_mode=DR)
                    dst = kT[t][:, spr * 1024:(spr + 1) * 1024].rearrange(
                        "p (k n) -> p k n", k=2)
                    drain_engine(2 * t + spr, dst, pk,
                                 bias=bk_sb[:, t:t + 1])

            # ---- Phase B: v[m, h, ci] = zf @ Wv, s-paired ----
            for spr in range(NMP):
                pv = ps_s.tile([128, 2, 512], F32, tag="s", name="pv")
                for k in range(2):
                    s = 2 * spr + k
                    nc.tensor.matmul(
                        pv[:, k, :],
                        z_sb[:, :, s * 128:(s + 1) * 128],
                        wv_sb[:, :, :],
                        start=True, stop=True, perf_mode=DR)
                drain_engine(spr + 1,
                             v_sb[:, 2 * spr:2 * spr + 2, :, 0:C],
                             pv.rearrange("p k (h c) -> p k h c", h=HEADS))

            # ---- denominator transpose chain, per l-half:
            # [8, 512] -> [l, 4, 8] via DVE 32x32 block transpose +
            # partition-shuffle DMAs + recip ----
            str_ = [cp.tile([32, 512], BF16, tag=f"str{i}", name=f"str{i}")
                    for i in range(2)]
            sums_t = cp.tile([128, NLS, 8], BF16, tag="sumst")
            recip_all = cp.tile([128, NLS, 8], F32, tag="recall")

            def emit_recip(lh):
                lo = lh * 512
                nc.vector.transpose(out=str_[lh], in_=sums_sb[:, lo:lo + 512])
                for j in range(4):
                    # split the partition-shuffle DMAs over both HW DGE
                    # queues: they are on the tail critical path
                    eng = nc.sync if j % 2 == 0 else nc.scalar
                    eng.dma_start(
                        out=sums_t[32 * j:32 * (j + 1), 4 * lh:4 * lh + 4, :],
                        in_=str_[lh].rearrange("p (ls j h32) -> p ls j h32",
                                               ls=4, j=4)[:, :, j, 0:8])
                nc.vector.reciprocal(out=recip_all[:, 4 * lh:4 * lh + 4, :],
                                     in_=sums_t[:, 4 * lh:4 * lh + 4, :])

            # ---- Phase D (per 128-row l-subtile): out-proj + normalize +
            # head-reduce + residual + store ----
            fin_all = cp.tile([128, NLS, C], F32, tag="finall")
            out_r = OUT.rearrange("(ls p) c -> p ls c", ls=NLS)

            def emit_po(ls):
                # po shares the score-psum ring so its matmuls only ever
                # wait on a 3-iterations-old exp, never on a far drain
                lo = ls * 128
                po = ps_s.tile([128, HEADS, C], F32, tag="s", name="po")
                for h in range(HEADS):
                    nc.tensor.matmul(
                        po[:, h, :],
                        pt_all[0:C, h, lo:lo + 128],
                        wo_sb[:, h, :],
                        start=True, stop=True)
                return po

            def emit_d(ls, po=None, tail=False):
                if po is None:
                    po = emit_po(ls)
                rb = recip_all[:, ls, :].rearrange("p (h o) -> p h o", o=1)
                fin = fin_all[:, ls, :]
                if tail:
                    # loop is over; DVE is idle: normalize + strided
                    # head-reduce there (Pool cannot read PSUM)
                    tmp = sp.tile([128, HEADS, C], F32, tag="tmp", name="tmp")
                    nc.vector.tensor_tensor(
                        out=tmp, in0=po,
                        in1=rb.broadcast_to([128, HEADS, C]), op=OP.mult)
                    red = sp.tile([128, C], F32, tag="red", name="red")
                    nc.vector.tensor_reduce(
                        out=red, in_=tmp.rearrange("p h c -> p c h"),
                        axis=mybir.AxisListType.X, op=OP.add)
                    nc.gpsimd.tensor_tensor(out=fin, in0=red,
                                            in1=xr_sb[:, ls, :], op=OP.add)
                else:
                    # mid-loop: one Act copy PSUM->SBUF, rest on idle Pool
                    po_sb = sp.tile([128, HEADS, C], F32, tag="posb",
                                    name="posb")
                    nc.scalar.activation(out=po_sb, in_=po, func=AF.Copy)
                    tmp = sp.tile([128, HEADS, C], F32, tag="tmp", name="tmp")
                    nc.gpsimd.tensor_tensor(
                        out=tmp, in0=po_sb,
                        in1=rb.broadcast_to([128, HEADS, C]), op=OP.mult)
                    t1 = sp.tile([128, 4, C], F32, tag="t1", name="t1")
                    nc.gpsimd.tensor_tensor(out=t1, in0=tmp[:, 0:4, :],
                                            in1=tmp[:, 4:8, :], op=OP.add)
                    t2 = sp.tile([128, 2, C], F32, tag="t2", name="t2")
                    nc.gpsimd.tensor_tensor(out=t2, in0=t1[:, 0:2, :],
                                            in1=t1[:, 2:4, :], op=OP.add)
                    nc.gpsimd.tensor_tensor(out=fin, in0=t2[:, 0, :],
                                            in1=t2[:, 1, :], op=OP.add)
                    nc.gpsimd.tensor_tensor(out=fin, in0=fin,
                                            in1=xr_sb[:, ls, :], op=OP.add)
                nc.sync.dma_start(out=out_r[:, ls, :], in_=fin_all[:, ls, :])

            # ---- Phase C: attention, software-pipelined (AV trails scores
            # by one m-pair so the exp latency hides) ----
            def emit_av(st):
                pt_e, pt_o, es, j, lh, p = st
                nc.tensor.matmul(
                    pt_e, v_sb[:, 2 * j:2 * j + 2, 2 * p, 0:C + 1],
                    es[:, 0, :, :], start=(j == 0),
                    stop=(j == NMP - 1), perf_mode=DR)
                nc.tensor.matmul(
                    pt_o, v_sb[:, 2 * j:2 * j + 2, 2 * p + 1, 0:C + 1],
                    es[:, 1, :, :], start=(j == 0),
                    stop=(j == NMP - 1), perf_mode=DR)

            def emit_drain(st):
                pt_e, pt_o, es, j, lh, p = st
                lo = lh * 512
                he, ho = 2 * p, 2 * p + 1
                nc.scalar.activation(out=pt_all[:, he, lo:lo + 512],
                                     in_=pt_e, func=AF.Copy)
                nc.scalar.activation(out=pt_all[:, ho, lo:lo + 512],
                                     in_=pt_o, func=AF.Copy)
                nc.sync.dma_start(out=sums_sb[he:he + 1, lo:lo + 512],
                                  in_=pt_all[C:C + 1, he, lo:lo + 512])
                nc.sync.dma_start(out=sums_sb[ho:ho + 1, lo:lo + 512],
                                  in_=pt_all[C:C + 1, ho, lo:lo + 512])
                if p == 3:
                    emit_recip(lh)
                # interleave the first half's phase D into the second
                # half's attention, two subtiles per drained head-pair
                if lh == 1 and p < 2:
                    emit_d(2 * p)
                    emit_d(2 * p + 1)

            pend = []
            it = 0
            for lh in range(2):
                lo = lh * 512
                for p in range(NP):
                    pt_e = ps_pt.tile([C + 1, 512], F32, tag="pt",
                                      name="pte")
                    pt_o = ps_pt.tile([C + 1, 512], F32, tag="pt",
                                      name="pto")
                    for j in range(NMP):
                        es = ep.tile([128, 2, 2, 512], F8, tag="es",
                                     name="es")
                        for k in range(2):
                            mt = 2 * j + k
                            ms = mt * 128
                            ps = ps_s.tile([128, 2, 512], F32, tag="s",
                                           name="ps")
                            nc.tensor.matmul(ps[:, 0, :],
                                             kT[p][0:64, ms:ms + 128],
                                             x_sb[0:64, lo:lo + 512],
                                             start=True, stop=True)
                            nc.tensor.matmul(ps[:, 1, :],
                                             kT[p][64:128, ms:ms + 128],
                                             x_sb[64:128, lo:lo + 512],
                                             start=True, stop=True)
                            e = EXP_ROT[it % len(EXP_ROT)]
                            if e == "A":
                                nc.scalar.activation(out=es[:, :, k, :],
                                                     in_=ps, func=AF.Exp,
                                                     scale=SCALE)
                            else:
                                eng = nc.vector if e == "D" else nc.gpsimd
                                eng.tensor_scalar(
                                    out=es[:, :, k, :].bitcast(I8), in0=ps,
                                    scalar1=EXP_A8, scalar2=EXP_B8,
                                    op0=OP.mult, op1=OP.add)
                            it += 1
                        pend.append((pt_e, pt_o, es, j, lh, p))
                        if len(pend) > TRAIL:
                            st = pend.pop(0)
                            emit_av(st)
                            if st[3] == NMP - 1:
                                emit_drain(st)
            for st in pend:
                emit_av(st)
                if st[3] == NMP - 1:
                    emit_drain(st)
            # tail: second half's out-proj subtiles. Emit the po matmul
            # groups first (they only need the pt drains), then the
            # normalize chains alternating DVE-path / Act+Pool-path so no
            # single engine serializes the tail.
            pos = [emit_po(4), emit_po(5), emit_po(6)]
            emit_d(4, po=pos[0], tail=True)
            pos.append(emit_po(7))
            emit_d(5, po=pos[1], tail=False)
            emit_d(6, po=pos[2], tail=True)
            emit_d(7, po=pos[3], tail=False)

    nc.compile()
    return nc


def kernel(x, z, Wk, bk, Wv, bv, Wo, bo):
    x = np.ascontiguousarray(x, dtype=np.float32)
    z = np.ascontiguousarray(z, dtype=np.float32)
    Wk = np.asarray(Wk, np.float32)
    Wv = np.asarray(Wv, np.float32)
    Wo = np.asarray(Wo, np.float32)
    bk = np.asarray(bk, np.float32)
    bv = np.asarray(bv, np.float32)
    bo = np.asarray(bo, np.float32)
    if "nc" not in _CACHE:
        _CACHE["nc"] = build_nc()
    nc = _CACHE["nc"]
    # out-proj is linear, so the constant V bias folds into the residual:
    # ((P + bv*d)/d) @ Wo + bo = (P/d) @ Wo + (bv @ Wo + bo)
    res_bias = bv @ Wo + bo                      # [C]

    def dr_layout(a):
        # [2*128, X] -> [128, 2, X]: the DoubleRow partition layout, so the
        # device DMA is one contiguous run per partition
        return np.ascontiguousarray(
            a.reshape(2, 128, a.shape[1]).transpose(1, 0, 2))

    shared = {
        "Wk": dr_layout(Wk.astype(mybir.dt.np(F8))),
        "Wv": dr_layout(Wv.astype(mybir.dt.np(F8))),
        "Wo": np.ascontiguousarray(
            Wo.reshape(HEADS, C, C).transpose(1, 0, 2)
            .astype(ml_dtypes.bfloat16)),
        "bk": np.ascontiguousarray(bk.reshape(4, 128).T),
    }
    in_maps = []
    for core in range(N_CORES):
        bi, half = core // 2, core % 2
        xi = x[bi].reshape(C, L)
        in_maps.append({
            "xq": np.ascontiguousarray(
                xi[:, half * LH:(half + 1) * LH].astype(ml_dtypes.bfloat16)),
            "xr": np.ascontiguousarray(
                x[bi].reshape(-1)[half * LH * C:(half + 1) * LH * C]
                .reshape(NLS, 128, C).transpose(1, 0, 2) + res_bias),
            "zb": dr_layout(z[bi].reshape(DIM, L).astype(mybir.dt.np(F8))),
            **shared,
        })
    _CACHE["in_maps"] = in_maps
    if "warm" not in _CACHE:
        # First execution after NEFF load runs with cold DMA rings and
        # wildly different timing; settle the device before the real run.
        run_bass_kernel_spmd(nc, in_maps, list(range(N_CORES)))
        _CACHE["warm"] = True
    res = run_bass_kernel_spmd(nc, in_maps, list(range(N_CORES)))
    full = np.empty((B, L * C), dtype=np.float32)
    for core in range(N_CORES):
        bi, half = core // 2, core % 2
        full[bi, half * LH * C:(half + 1) * LH * C] = \
            res.results[core]["out"].reshape(-1)
    return full.reshape(B, C, H, W)


# revision 24
# speedup vs baseline: 1.1799x; 1.1799x over previous
"""CrossAttnBlock TRN2 kernel: 8-way (batch x l-half) sharded, collective-free.

Reference math (b=4, c=64, h=64, w=32, dim=256, HEADS=8, l=h*w=2048):
  zf = z.reshape(b, dim, l).T            # [b, l, dim]
  q  = x.reshape(b, c, l).T              # [b, l, c]
  k  = (zf @ Wk + bk) -> [b, H, l, c];  v likewise
  S  = q @ k.T / sqrt(c); A = softmax(S, -1); P = A @ v
  out = (P heads-concat) @ Wo + bo       # [b, l, c]
  return x + out.reshape(b, c, h, w)     # raw-memory reinterpretation

Per-core (core = bi*2 + half): full K/V projection for batch bi, attention +
out-proj for l rows [half*1024, (half+1)*1024).

v2 structure (vs v1): score matmuls write paired 2-bank PSUM tiles so ONE
exp op per iteration covers both heads of the pair; the exp rotates across
Act / Pool / DVE (weighted) instead of saturating Act+DVE only. Phase A/B
drains are paired the same way. Phase D (normalize via recip broadcast +
strided head-reduce + residual) is emitted per-128-row subtile, interleaved
into the second l-half's attention so the tail is only the last 4 subtiles.
Input DMAs are chunked and split across the SP and Act DGE queues so phase A
starts ~1.5us in. Softmax denominators come from a ones-augmented V column
(memset, not DMA'd). bo and bv fold into the host-side residual; bk is added
on the kT drain path.
"""
import ml_dtypes
import numpy as np

import concourse.bass as bass
import concourse.mybir as mybir
import concourse.tile as tile
from concourse import bacc
from concourse.bass_utils import run_bass_kernel_spmd

F32 = mybir.dt.float32
BF16 = mybir.dt.bfloat16
I16 = mybir.dt.int16
I8 = mybir.dt.int8
F8 = mybir.dt.float8e4

B, C, H, W = 4, 64, 64, 32
DIM = 256
HEADS = 8
L = H * W            # 2048
LH = L // 2          # 1024 per core
INNER = HEADS * C    # 512
N_CORES = 8
NMT = L // 128       # 16 m-tiles
NMP = NMT // 2       # 8 m-tile pairs
NLS = LH // 128      # 8 l-subtiles
NP = HEADS // 2      # 4 head pairs

SCALE = float(C) ** -0.5
# Schraudolph exp in fp8e4m3 bits: fp8(exp(s*SCALE)) ~ int8(s*EXP_A8 + EXP_B8)
EXP_A8 = (2.0 ** 3 / float(np.log(2.0))) * SCALE
EXP_B8 = 56.0 - 0.46

# exp engine per iteration (mod len): A=Act table exp, D=DVE schraudolph.
# (GPSIMD/Pool cannot access PSUM, so it only gets SBUF-side phase-D work.)
# Ratio tuned from measured pair-op costs: Act 1058ns vs DVE 1225ns, with
# Act also carrying the pt drains + po copies and DVE the A/B drains.
EXP_ROT = "AD"
# AV matmuls trail the score matmuls by this many m-tile pairs so the PE
# never waits on a just-issued exp (3: also covers the section-boundary
# pt-drain -> next-section-AV handoff)
TRAIL = 3

_CACHE = {}


def build_nc():
    nc = bacc.Bacc("TRN2", target_bir_lowering=False, debug=False,
                   num_devices=N_CORES)
    # weights/activations arrive host-pre-interleaved in the DoubleRow
    # [partition, row-pair, col] layout so each DMA is one contiguous run
    # per partition (128 big descriptors instead of 256 small ones)
    zb = nc.dram_tensor("zb", [128, 2, L], F8, kind="ExternalInput")
    Wk = nc.dram_tensor("Wk", [128, 2, INNER], F8, kind="ExternalInput")
    Wv = nc.dram_tensor("Wv", [128, 2, INNER], F8, kind="ExternalInput")
    xq = nc.dram_tensor("xq", [C, LH], BF16, kind="ExternalInput")
    xr = nc.dram_tensor("xr", [128, NLS, C], F32, kind="ExternalInput")
    Wo = nc.dram_tensor("Wo", [C, HEADS, C], BF16, kind="ExternalInput")
    bk = nc.dram_tensor("bk", [128, 4], F32, kind="ExternalInput")
    OUT = nc.dram_tensor("out", [LH, C], F32, kind="ExternalOutput")

    AF = mybir.ActivationFunctionType
    OP = mybir.AluOpType
    DR = mybir.MatmulPerfMode.DoubleRow

    with tile.TileContext(nc) as tc:
        with (
            tc.tile_pool(name="const", bufs=1) as cp,
            tc.tile_pool(name="es", bufs=6) as ep,
            tc.tile_pool(name="sm", bufs=3) as sp,
            tc.tile_pool(name="ps_s", bufs=3, space="PSUM") as ps_s,
            tc.tile_pool(name="ps_pt", bufs=2, space="PSUM") as ps_pt,
        ):
            # ---- inputs to SBUF. sync (SP) queue: wk whole (first matmul
            # needs it), z whole (contiguous beats chunked: one 630ns
            # trigger, full-rate transfer), bk, wv.
            wk_sb = cp.tile([128, 2, INNER], F8, tag="wk")
            nc.sync.dma_start(out=wk_sb, in_=Wk[:, :, :])
            z_sb = cp.tile([128, 2, L], F8, tag="z")
            nc.sync.dma_start(out=z_sb, in_=zb[:, :, :])
            bk_sb = cp.tile([128, 4], F32, tag="bk")
            nc.sync.dma_start(out=bk_sb, in_=bk[:, :])
            wv_sb = cp.tile([128, 2, INNER], F8, tag="wv")
            nc.sync.dma_start(out=wv_sb, in_=Wv[:, :, :])
            # scalar (Act) queue: x (q), residual, Wo
            x_sb = cp.tile([128, LH], BF16, tag="x")
            nc.scalar.dma_start(out=x_sb[0:C, :], in_=xq[:, :])
            nc.scalar.dma_start(out=x_sb[C:2 * C, :], in_=xq[:, :])
            xr_sb = cp.tile([128, NLS, C], F32, tag="xr")
            nc.scalar.dma_start(out=xr_sb, in_=xr[:, :, :])
            wo_sb = cp.tile([C, HEADS, C], BF16, tag="wo")
            nc.scalar.dma_start(out=wo_sb, in_=Wo[:, :, :])

            # warm-up matmuls on a memset tile: no input deps, so they run
            # during the DMA wait and carry the PE p-state ramp (0.65 ->
            # 2.4GHz after ~3us continuous) before phase A starts
            warm = cp.tile([128, 512], BF16, tag="warm")
            nc.gpsimd.memset(warm, 0.0)
            for _ in range(4):
                pw = ps_s.tile([128, 2, 512], F32, tag="s", name="pw")
                nc.tensor.matmul(pw[:, 0, :], warm[:, 0:128], warm,
                                 start=True, stop=True)

            kT = [cp.tile([128, L], BF16, tag=f"kT{t}", name=f"kT{t}")
                  for t in range(4)]
            v_sb = cp.tile([128, NMT, HEADS, C + 2], F8, tag="v")
            nc.gpsimd.memset(v_sb[:, :, :, C:C + 1], 1.0)
            # P^T numerators for all heads: [c+1, head, l]; row C holds the
            # softmax denominators
            pt_all = cp.tile([C + 1, HEADS, LH], BF16, tag="pt")
            # denominators, rows 0-7 used; padded to 32 partitions for the
            # DVE block transpose
            sums_sb = cp.tile([32, LH], BF16, tag="sums")
            nc.gpsimd.memset(sums_sb, 1.0)

            def drain_engine(idx, out, in_, bias=None):
                # alternate Act/DVE: both are otherwise idle in the prologue
                if bias is not None:
                    if idx % 2 == 0:
                        nc.scalar.activation(out=out, in_=in_,
                                             func=AF.Identity, bias=bias,
                                             scale=1.0)
                    else:
                        nc.vector.tensor_scalar(out=out, in0=in_,
                                                scalar1=bias, scalar2=None,
                                                op0=OP.add)
                else:
                    if idx % 2 == 0:
                        nc.scalar.activation(out=out, in_=in_, func=AF.Copy)
                    else:
                        nc.vector.tensor_copy(out=out, in_=in_)

            # ---- Phase A: kT[t][ci, m] = (Wk^T @ zf^T) + bk, s-paired ----
            for t in range(4):
                for spr in range(2):
                    pk = ps_s.tile([128, 2, 512], F32, tag="s", name="pk")
                    for k in range(2):
                        s = 2 * spr + k
                        nc.tensor.matmul(
                            pk[:, k, :],
                            wk_sb[:, :, t * 128:(t + 1) * 128],
                            z_sb[:, :, s * 512:(s + 1) * 512],
                            start=True, stop=True, perf_mode=DR)
                    dst = kT[t][:, spr * 1024:(spr + 1) * 1024].rearrange(
                        "p (k n) -> p k n", k=2)
                    drain_engine(2 * t + spr, dst, pk,
                                 bias=bk_sb[:, t:t + 1])

            # ---- Phase B: v[m, h, ci] = zf @ Wv, s-paired ----
            for spr in range(NMP):
                pv = ps_s.tile([128, 2, 512], F32, tag="s", name="pv")
                for k in range(2):
                    s = 2 * spr + k
                    nc.tensor.matmul(
                        pv[:, k, :],
                        z_sb[:, :, s * 128:(s + 1) * 128],
                        wv_sb[:, :, :],
                        start=True, stop=True, perf_mode=DR)
                drain_engine(spr + 1,
                             v_sb[:, 2 * spr:2 * spr + 2, :, 0:C],
                             pv.rearrange("p k (h c) -> p k h c", h=HEADS))

            # ---- denominator transpose chain, per l-half:
            # [8, 512] -> [l, 4, 8] via DVE 32x32 block transpose +
            # partition-shuffle DMAs + recip ----
            str_ = [cp.tile([32, 512], BF16, tag=f"str{i}", name=f"str{i}")
                    for i in range(2)]
            sums_t = cp.tile([128, NLS, 8], BF16, tag="sumst")
            recip_all = cp.tile([128, NLS, 8], F32, tag="recall")

            def emit_recip(lh):
                lo = lh * 512
                nc.vector.transpose(out=str_[lh], in_=sums_sb[:, lo:lo + 512])
                for j in range(4):
                    # split the partition-shuffle DMAs over both HW DGE
                    # queues: they are on the tail critical path
                    eng = nc.sync if j % 2 == 0 else nc.scalar
                    eng.dma_start(
                        out=sums_t[32 * j:32 * (j + 1), 4 * lh:4 * lh + 4, :],
                        in_=str_[lh].rearrange("p (ls j h32) -> p ls j h32",
                                               ls=4, j=4)[:, :, j, 0:8])
                nc.vector.reciprocal(out=recip_all[:, 4 * lh:4 * lh + 4, :],
                                     in_=sums_t[:, 4 * lh:4 * lh + 4, :])

            # ---- Phase D (per 128-row l-subtile): out-proj + normalize +
            # head-reduce + residual + store ----
            fin_all = cp.tile([128, NLS, C], F32, tag="finall")
            out_r = OUT.rearrange("(ls p) c -> p ls c", ls=NLS)

            def emit_po(ls):
                # po shares the score-psum ring so its matmuls only ever
                # wait on a 3-iterations-old exp, never on a far drain
                lo = ls * 128
                po = ps_s.tile([128, HEADS, C], F32, tag="s", name="po")
                for h in range(HEADS):
                    nc.tensor.matmul(
                        po[:, h, :],
                        pt_all[0:C, h, lo:lo + 128],
                        wo_sb[:, h, :],
                        start=True, stop=True)
                return po

            def emit_d(ls, po=None, tail=False):
                if po is None:
                    po = emit_po(ls)
                rb = recip_all[:, ls, :].rearrange("p (h o) -> p h o", o=1)
                fin = fin_all[:, ls, :]
                if tail:
                    # loop is over; DVE is idle: normalize + strided
                    # head-reduce there (Pool cannot read PSUM)
                    tmp = sp.tile([128, HEADS, C], F32, tag="tmp", name="tmp")
                    nc.vector.tensor_tensor(
                        out=tmp, in0=po,
                        in1=rb.broadcast_to([128, HEADS, C]), op=OP.mult)
                    red = sp.tile([128, C], F32, tag="red", name="red")
                    nc.vector.tensor_reduce(
                        out=red, in_=tmp.rearrange("p h c -> p c h"),
                        axis=mybir.AxisListType.X, op=OP.add)
                    nc.gpsimd.tensor_tensor(out=fin, in0=red,
                                            in1=xr_sb[:, ls, :], op=OP.add)
                else:
                    # mid-loop: one Act copy PSUM->SBUF, rest on idle Pool
                    po_sb = sp.tile([128, HEADS, C], F32, tag="posb",
                                    name="posb")
                    nc.scalar.activation(out=po_sb, in_=po, func=AF.Copy)
                    tmp = sp.tile([128, HEADS, C], F32, tag="tmp", name="tmp")
                    nc.gpsimd.tensor_tensor(
                        out=tmp, in0=po_sb,
                        in1=rb.broadcast_to([128, HEADS, C]), op=OP.mult)
                    t1 = sp.tile([128, 4, C], F32, tag="t1", name="t1")
                    nc.gpsimd.tensor_tensor(out=t1, in0=tmp[:, 0:4, :],
                                            in1=tmp[:, 4:8, :], op=OP.add)
                    t2 = sp.tile([128, 2, C], F32, tag="t2", name="t2")
                    nc.gpsimd.tensor_tensor(out=t2, in0=t1[:, 0:2, :],
                                            in1=t1[:, 2:4, :], op=OP.add)
                    nc.gpsimd.tensor_tensor(out=fin, in0=t2[:, 0, :],
                                            in1=t2[:, 1, :], op=OP.add)
                    nc.gpsimd.tensor_tensor(out=fin, in0=fin,
                                            in1=xr_sb[:, ls, :], op=OP.add)
                nc.sync.dma_start(out=out_r[:, ls, :], in_=fin_all[:, ls, :])

            # ---- Phase C: attention, software-pipelined (AV trails scores
            # by one m-pair so the exp latency hides) ----
            def emit_av(st):
                pt_e, pt_o, es, j, lh, p = st
                nc.tensor.matmul(
                    pt_e, v_sb[:, 2 * j:2 * j + 2, 2 * p, 0:C + 1],
                    es[:, 0, :, :], start=(j == 0),
                    stop=(j == NMP - 1), perf_mode=DR)
                nc.tensor.matmul(
                    pt_o, v_sb[:, 2 * j:2 * j + 2, 2 * p + 1, 0:C + 1],
                    es[:, 1, :, :], start=(j == 0),
                    stop=(j == NMP - 1), perf_mode=DR)

            def emit_drain(st):
                pt_e, pt_o, es, j, lh, p = st
                lo = lh * 512
                he, ho = 2 * p, 2 * p + 1
                nc.scalar.activation(out=pt_all[:, he, lo:lo + 512],
                                     in_=pt_e, func=AF.Copy)
                nc.scalar.activation(out=pt_all[:, ho, lo:lo + 512],
                                     in_=pt_o, func=AF.Copy)
                nc.sync.dma_start(out=sums_sb[he:he + 1, lo:lo + 512],
                                  in_=pt_all[C:C + 1, he, lo:lo + 512])
                nc.sync.dma_start(out=sums_sb[ho:ho + 1, lo:lo + 512],
                                  in_=pt_all[C:C + 1, ho, lo:lo + 512])
                if p == 3:
                    emit_recip(lh)
                # interleave the first half's phase D into the second
                # half's attention, two subtiles per drained head-pair
                if lh == 1 and p < 2:
                    emit_d(2 * p)
                    emit_d(2 * p + 1)

            pend = []
            it = 0
            for lh in range(2):
                lo = lh * 512
                for p in range(NP):
                    pt_e = ps_pt.tile([C + 1, 512], F32, tag="pt",
                                      name="pte")
                    pt_o = ps_pt.tile([C + 1, 512], F32, tag="pt",
                                      name="pto")
                    for j in range(NMP):
                        es = ep.tile([128, 2, 2, 512], F8, tag="es",
                                     name="es")
                        for k in range(2):
                            mt = 2 * j + k
                            ms = mt * 128
                            ps = ps_s.tile([128, 2, 512], F32, tag="s",
                                           name="ps")
                            nc.tensor.matmul(ps[:, 0, :],
                                             kT[p][0:64, ms:ms + 128],
                                             x_sb[0:64, lo:lo + 512],
                                             start=True, stop=True)
                            nc.tensor.matmul(ps[:, 1, :],
                                             kT[p][64:128, ms:ms + 128],
                                             x_sb[64:128, lo:lo + 512],
                                             start=True, stop=True)
                            e = EXP_ROT[it % len(EXP_ROT)]
                            if e == "A":
                                nc.scalar.activation(out=es[:, :, k, :],
                                                     in_=ps, func=AF.Exp,
                                                     scale=SCALE)
                            else:
                                eng = nc.vector if e == "D" else nc.gpsimd
                                eng.tensor_scalar(
                                    out=es[:, :, k, :].bitcast(I8), in0=ps,
                                    scalar1=EXP_A8, scalar2=EXP_B8,
                                    op0=OP.mult, op1=OP.add)
                            it += 1
                        pend.append((pt_e, pt_o, es, j, lh, p))
                        if len(pend) > TRAIL:
                            st = pend.pop(0)
                            emit_av(st)
                            if st[3] == NMP - 1:
                                emit_drain(st)
            for st in pend:
                emit_av(st)
                if st[3] == NMP - 1:
                    emit_drain(st)
            # tail: second half's out-proj subtiles. Emit the po matmul
            # groups first (they only need the pt drains), then the
            # normalize chains alternating DVE-path / Act+Pool-path so no
            # single engine serializes the tail.
            pos = [emit_po(4), emit_po(5), emit_po(6)]
            emit_d(4, po=pos[0], tail=True)
            pos.append(emit_po(7))
            emit_d(5, po=pos[1], tail=False)
            emit_d(6, po=pos[2], tail=True)
            emit_d(7, po=pos[3], tail=False)

    nc.compile()
    return nc


def kernel(x, z, Wk, bk, Wv, bv, Wo, bo):
    x = np.ascontiguousarray(x, dtype=np.float32)
    z = np.ascontiguousarray(z, dtype=np.float32)
    Wk = np.asarray(Wk, np.float32)
    Wv = np.asarray(Wv, np.float32)
    Wo = np.asarray(Wo, np.float32)
    bk = np.asarray(bk, np.float32)
    bv = np.asarray(bv, np.float32)
    bo = np.asarray(bo, np.float32)
    if "nc" not in _CACHE:
        _CACHE["nc"] = build_nc()
    nc = _CACHE["nc"]
    # out-proj is linear, so the constant V bias folds into the residual:
    # ((P + bv*d)/d) @ Wo + bo = (P/d) @ Wo + (bv @ Wo + bo)
    res_bias = bv @ Wo + bo                      # [C]

    def dr_layout(a):
        # [2*128, X] -> [128, 2, X]: the DoubleRow partition layout, so the
        # device DMA is one contiguous run per partition
        return np.ascontiguousarray(
            a.reshape(2, 128, a.shape[1]).transpose(1, 0, 2))

    shared = {
        "Wk": dr_layout(Wk.astype(mybir.dt.np(F8))),
        "Wv": dr_layout(Wv.astype(mybir.dt.np(F8))),
        "Wo": np.ascontiguousarray(
            Wo.reshape(HEADS, C, C).transpose(1, 0, 2)
            .astype(ml_dtypes.bfloat16)),
        "bk": np.ascontiguousarray(bk.reshape(4, 128).T),
    }
    in_maps = []
    for core in range(N_CORES):
        bi, half = core // 2, core % 2
        xi = x[bi].reshape(C, L)
        in_maps.append({
            "xq": np.ascontiguousarray(
                xi[:, half * LH:(half + 1) * LH].astype(ml_dtypes.bfloat16)),
            "xr": np.ascontiguousarray(
                x[bi].reshape(-1)[half * LH * C:(half + 1) * LH * C]
                .reshape(NLS, 128, C).transpose(1, 0, 2) + res_bias),
            "zb": dr_layout(z[bi].reshape(DIM, L).astype(mybir.dt.np(F8))),
            **shared,
        })
    _CACHE["in_maps"] = in_maps
    if "warm" not in _CACHE:
        # First execution after NEFF load runs with cold DMA rings and
        # wildly different timing; settle the device before the real run.
        run_bass_kernel_spmd(nc, in_maps, list(range(N_CORES)))
        _CACHE["warm"] = True
    res = run_bass_kernel_spmd(nc, in_maps, list(range(N_CORES)))
    full = np.empty((B, L * C), dtype=np.float32)
    for core in range(N_CORES):
        bi, half = core // 2, core % 2
        full[bi, half * LH * C:(half + 1) * LH * C] = \
            res.results[core]["out"].reshape(-1)
    return full.reshape(B, C, H, W)
